# revision 31
# baseline (speedup 1.0000x reference)
"""Trainium2 Bass kernel for EmbededNonLocalLayer (fp8 DoubleRow version).

Distribution: 8 cores = 4 batches x 2 query-halves. Each core holds its
batch's full keys; its query half sits at columns [0:2048) of a rolled x.

Math (per core), with host scales SK=16 (qk path), SW=64 (w2 path),
SS=32 (r1 ones), SS2=1024 (centered simv):
  qk8   = fp8(wk8^T x8 + bk2*SK)              [256, 4096]  (conv, DoubleRow)
  v2    = Wv2 @ (Wv @ xpool)/49               [256, 82]    (fp32, col0=0)
  w28   = fp8(Wv^T v2 * SW)                   [512, 82]    (val^T v2 == x^T w2)
  lgt2  = x8^T w28                            per 128-key block (DoubleRow)
  simv  = softmax_k(lgt2 * S/SW); dsimv8 = fp8((simv - 1/81)*SS2), col0 = SS
  E8    = fp8(exp(qk8^T qk8 * S/SK^2))        ACT exp or DVE Schraudolph bits
  o82   = dsimv8^T E8  (DoubleRow, PSUM accum over 32 key blocks)
          row0 = SS*r1 (softmax denom), rows 1:82 = SS2 * (dsimv^T E)
  ctx   = (v2t^T o82) * (1/row0 bcast);  out = (Ww*SS/SS2)^T ctx + wu^T x 1s
          (wu = Ww @ v2.sum/81 restores the centered-simv mean term)
"""

import sys

sys.path.insert(0, "/opt/trn_rl_repo")

import numpy as np
import ml_dtypes

import concourse.bacc as bacc
import concourse.bass as bass
import concourse.mybir as mybir
from concourse.bass_utils import run_bass_kernel_spmd
from concourse.tile import TileContext

F32 = mybir.dt.float32
F32R = mybir.dt.float32r
FP8 = mybir.dt.float8e4
U8 = mybir.dt.uint8
AF = mybir.ActivationFunctionType
AX = mybir.AxisListType
DR = mybir.MatmulPerfMode.DoubleRow
OP = mybir.AluOpType
NPF8 = ml_dtypes.float8_e4m3

B, CIN, H, W = 4, 512, 63, 63
N = H * W            # 3969
NPAD = 4096
CI, CO = 256, 512
KK = 81
SCALE = 0.0625       # 1/sqrt(CI)
QCNT = 1985
QP = 2048
Q0STEP = 1984
MB = NPAD // 128     # 32 key blocks
SLABS = 8            # 512-column x slabs

SK = 16.0            # qk fp8 scale
SW = 64.0            # w2 fp8 scale
SS = 32.0            # ones column scale (r1 row)
SS2 = 1024.0         # centered-simv scale
EXP_SCALE = SCALE / (SK * SK)
LG2_SCALE = SCALE / SW
SIGMA = 0.35
A_SCH = 8.0 / np.log(2.0) * EXP_SCALE
B_SCH = 8.0 * 7.0 + SIGMA
A16_SCH = 128.0 / np.log(2.0) * LG2_SCALE
B16_SCH = 128.0 * 127.0 + 3.0

# query-column widths per (qp, h): qp1's second half holds only the 449
# real query columns (1985 total); the out tail beyond QCNT is never read.
WH = {(0, 0): 512, (0, 1): 512, (1, 0): 512, (1, 1): 450}
QW = {0: 1024, 1: 962}

# (qp, mb) units whose exp runs on DVE via Schraudolph bits; rest on ACT.
DVE_EXP = {(qp, mb) for qp in range(2) for mb in range(MB)
           if mb % 5 in (1, 3)}

_CACHE = {}


def _build_program(reps=1):
    nc = bacc.Bacc()

    x8_d = nc.dram_tensor("x8", [CIN, N], FP8, kind="ExternalInput")
    xp_d = nc.dram_tensor("xpool", [CIN, 82], F32R, kind="ExternalInput")
    wk8_d = nc.dram_tensor("wk8", [128, 1024], FP8, kind="ExternalInput")
    wv_d = nc.dram_tensor("wvT", [CIN, CI], F32R, kind="ExternalInput")
    wv2_d = nc.dram_tensor("wv2T", [CI, CI], F32R, kind="ExternalInput")
    wvO_d = nc.dram_tensor("wvO", [CI, CIN], F32R, kind="ExternalInput")
    wws_d = nc.dram_tensor("wws", [CI, CO], F32R, kind="ExternalInput")
    bk2s_d = nc.dram_tensor("bk2s", [128, 4], F32, kind="ExternalInput")
    ones_d = nc.dram_tensor("ones1", [1, 512], F32R, kind="ExternalInput")
    c8_d = nc.dram_tensor("c8ones", [128, 256], FP8, kind="ExternalInput")
    fz_d = nc.dram_tensor("fzero", [128, 8], F32R, kind="ExternalInput")
    out_d = nc.dram_tensor("out", [CO, QP], F32, kind="ExternalOutput")

    with TileContext(nc) as tc, \
         nc.allow_low_precision(reason="fp8 attention validated numerically"):
      for _rep in range(reps):
        with tc.tile_pool(name=f"const{_rep}", bufs=1) as cpool, \
             tc.tile_pool(name=f"work{_rep}", bufs=1) as wpool:
          ones_sb = cpool.tile([1, 512], F32R)
          wk8_sb = cpool.tile([128, 1024], FP8)
          wv_sb = cpool.tile([128, 4 * CI], F32R)
          wv2_sb = cpool.tile([128, 2 * CI], F32R)
          wvO_sb = cpool.tile([128, 2 * CIN], F32R)
          wws_sb = cpool.tile([128, 2 * CO], F32R)
          bk2s_sb = cpool.tile([128, 4], F32)
          xp_sb = cpool.tile([128, 4 * 82], F32R)
          x8_sb = cpool.tile([128, 4 * NPAD], FP8)
          qk8_sb = cpool.tile([128, 2 * NPAD], FP8)
          dsimv8_sb = cpool.tile([128, MB * 96], FP8)
          pooled_sb = cpool.tile([128, 2 * 82], F32R)
          v2_sb = cpool.tile([128, 2 * 82], F32R)
          v2t_sb = cpool.tile([82, CI], F32R)
          w28_sb = cpool.tile([128, 4 * 96], FP8)
          r2_sb = cpool.tile([128, MB], F32)
          r2i2_sb = cpool.tile([128, MB], F32)
          v2s_sb = cpool.tile([128, 4], F32R)
          wu_sb = cpool.tile([1, CO], F32R)

          x8_4 = x8_sb.rearrange("p (c n) -> p c n", c=4)
          qk3 = qk8_sb.rearrange("p (t n) -> p t n", t=2)
          w28_4 = w28_sb.rearrange("p (c k) -> p c k", c=4)  # k=96
          dsim3 = dsimv8_sb.rearrange("p (m c) -> p m c", m=MB)  # c=96

          # ---------- emission helpers ----------
          def _slab_dma(s):
              n0 = s * 512
              rl = min(512, N - n0)
              nc.sync.dma_start(
                  out=x8_4[:, :, n0:n0 + rl],
                  in_=x8_d.rearrange("(c p) n -> p c n", c=4)[:, :, n0:n0 + rl])

          def emit_loads_early():
              nc.sync.dma_start(out=wk8_sb[:], in_=wk8_d[:])
              _slab_dma(0)
              _slab_dma(1)
              nc.sync.dma_start(out=bk2s_sb[:], in_=bk2s_d[:])
              _slab_dma(2)
              _slab_dma(3)
              nc.sync.dma_start(
                  out=xp_sb.rearrange("p (c k) -> p c k", c=4),
                  in_=xp_d.rearrange("(c p) k -> p c k", c=4))
              nc.sync.dma_start(
                  out=wv_sb.rearrange("p (c k) -> p c k", c=4),
                  in_=wv_d.rearrange("(c p) k -> p c k", c=4))
              nc.sync.dma_start(
                  out=wv2_sb.rearrange("p (c k) -> p c k", c=2),
                  in_=wv2_d.rearrange("(c p) k -> p c k", c=2))
              nc.sync.dma_start(
                  out=wvO_sb.rearrange("p (c k) -> p c k", c=2),
                  in_=wvO_d.rearrange("(c p) k -> p c k", c=2))
              nc.sync.dma_start(
                  out=wws_sb.rearrange("p (c k) -> p c k", c=2),
                  in_=wws_d.rearrange("(c p) k -> p c k", c=2))
              nc.sync.dma_start(out=ones_sb[:], in_=ones_d[:])
              nc.sync.dma_start(
                  out=dsim3[:, :, 0:1],
                  in_=c8_d[:, 0:MB].rearrange("p (m c) -> p m c", m=MB))
              for s in range(4, 8):
                  _slab_dma(s)
              for cc in range(4):
                  nc.sync.dma_start(
                      out=x8_4[:, cc:cc + 1, N:NPAD],
                      in_=c8_d[:, 128:128 + (NPAD - N)].rearrange(
                          "p (o n) -> p o n", o=1))

          def emit_conv(psH, s):
              n0 = s * 512
              for blk in range(2):
                  ps = psH.tile([128, 512], F32, tag="ps", name=f"cv{s}_{blk}")
                  for pr in range(2):
                      lhsT = wk8_sb[:, blk * 512 + pr * 256:
                                    blk * 512 + pr * 256 + 256].rearrange(
                          "p (s o) -> p s o", s=2)
                      nc.tensor.matmul(
                          ps[:], lhsT,
                          x8_4[:, 2 * pr:2 * pr + 2, n0:n0 + 512],
                          start=(pr == 0), stop=(pr == 1), perf_mode=DR)
                  qslice = qk8_sb[:, blk * NPAD + n0:blk * NPAD + n0 + 512]
                  if blk == 0:
                      nc.scalar.activation(qslice, ps[:], AF.Identity,
                                           bias=bk2s_sb[:, blk:blk + 1])
                  else:
                      nc.vector.tensor_scalar(
                          out=qslice, in0=ps[:],
                          scalar1=bk2s_sb[:, blk:blk + 1], scalar2=None,
                          op0=OP.add)

          def emit_pooled_path(psH):
              for blk in range(2):
                  ps = psH.tile([128, 512], F32, tag="ps", name=f"pooled{blk}")
                  for cc in range(4):
                      nc.tensor.matmul(
                          ps[:, :82],
                          wv_sb[:, cc * CI + blk * 128:
                                cc * CI + blk * 128 + 128],
                          xp_sb[:, cc * 82:(cc + 1) * 82],
                          start=(cc == 0), stop=(cc == 3))
                  nc.vector.tensor_copy(pooled_sb[:, blk * 82:(blk + 1) * 82],
                                        ps[:, :82])
              for blk in range(2):
                  ps = psH.tile([128, 512], F32, tag="ps", name=f"v2_{blk}")
                  for cc in range(2):
                      nc.tensor.matmul(
                          ps[:, :82],
                          wv2_sb[:, cc * CI + blk * 128:
                                 cc * CI + blk * 128 + 128],
                          pooled_sb[:, cc * 82:(cc + 1) * 82],
                          start=(cc == 0), stop=(cc == 1))
                  nc.vector.tensor_copy(v2_sb[:, blk * 82:(blk + 1) * 82],
                                        ps[:, :82])
              # w2 = Wv^T v2 (contract ci), scaled into fp8
              for oc4 in range(4):
                  ps = psH.tile([128, 512], F32, tag="ps", name=f"w2_{oc4}")
                  for cc in range(2):
                      nc.tensor.matmul(
                          ps[:, :82],
                          wvO_sb[:, cc * CIN + oc4 * 128:
                                 cc * CIN + oc4 * 128 + 128],
                          v2_sb[:, cc * 82:(cc + 1) * 82],
                          start=(cc == 0), stop=(cc == 1))
                  nc.vector.tensor_scalar(
                      out=w28_sb[:, oc4 * 96:oc4 * 96 + 82], in0=ps[:, :82],
                      scalar1=SW, scalar2=None, op0=OP.mult)

          def emit_tail_consts(psH):
              ps = psH.tile([128, 512], F32, tag="ps", name="v2t")
              for cc in range(2):
                  nc.tensor.matmul(ps[:82, :CI],
                                   pooled_sb[:, cc * 82:(cc + 1) * 82],
                                   wv2_sb[:, cc * CI:(cc + 1) * CI],
                                   start=(cc == 0), stop=(cc == 1))
              nc.vector.tensor_copy(v2t_sb[:], ps[:82, :CI])
              # v2s = rowsum(v2) * SS2/(SS*81); wu = v2s^T wws  -> [1, CO]
              # v2s is stored interleaved [c0, 0, c1, 0] so the wu matmul's
              # stationary free extent is 2 (fp32r needs an even count)
              nc.sync.dma_start(out=v2s_sb[:], in_=fz_d[:, 4:8])
              nc.vector.reduce_sum(
                  v2s_sb.rearrange("p (c z) -> p c z", c=2)[:, :, 0:1],
                  v2_sb.rearrange("p (c k) -> p c k", c=2),
                  axis=AX.X)
              nc.vector.tensor_scalar(
                  out=v2s_sb[:], in0=v2s_sb[:],
                  scalar1=float(SS2 / (SS * 81.0)), scalar2=None, op0=OP.mult)
              ps = psH.tile([128, 512], F32, tag="ps", name="wu")
              for cc in range(2):
                  nc.tensor.matmul(ps[0:2, :CO],
                                   v2s_sb[:, 2 * cc:2 * cc + 2],
                                   wws_sb[:, cc * CO:(cc + 1) * CO],
                                   start=(cc == 0), stop=(cc == 1))
              nc.vector.tensor_copy(wu_sb[:], ps[0:1, :CO])

          def emit_lg2(psH, s):
              ps2 = psH.tile([128, 512], F32, tag="ps", name=f"lg{s}")
              for j in range(4):
                  mb = 4 * s + j
                  for pr in range(2):
                      nc.tensor.matmul(
                          ps2[:, j * 82:j * 82 + 82],
                          x8_4[:, 2 * pr:2 * pr + 2, mb * 128:mb * 128 + 128],
                          w28_4[:, 2 * pr:2 * pr + 2, 0:82],
                          start=(pr == 0), stop=(pr == 1), perf_mode=DR)
              ex2 = wpool.tile([128, 4 * 82], F32, tag="ex2", bufs=2,
                               name=f"ex2_{s}")
              ps2v = ps2[:, 0:328].rearrange("p (g c) -> p g c", g=4)
              ex2v = ex2.rearrange("p (g c) -> p g c", g=4)
              nc.scalar.activation(ex2v[:, :, 1:82], ps2v[:, :, 1:82],
                                   AF.Exp, scale=LG2_SCALE)
              nc.vector.reduce_sum(r2_sb[:, 4 * s:4 * s + 4],
                                   ex2v[:, :, 1:82], axis=AX.X)
              nc.vector.reciprocal(r2i2_sb[:, 4 * s:4 * s + 4],
                                   r2_sb[:, 4 * s:4 * s + 4])
              nc.vector.tensor_scalar(
                  out=r2i2_sb[:, 4 * s:4 * s + 4],
                  in0=r2i2_sb[:, 4 * s:4 * s + 4],
                  scalar1=SS2, scalar2=None, op0=OP.mult)
              for j in range(4):
                  mb = 4 * s + j
                  nc.gpsimd.tensor_scalar(
                      out=dsim3[:, mb:mb + 1, 1:82],
                      in0=ex2v[:, j:j + 1, 1:82],
                      scalar1=r2i2_sb[:, mb:mb + 1],
                      scalar2=float(SS2 / 81.0),
                      op0=OP.mult, op1=OP.subtract)

          def emit_mask():
              nc.gpsimd.tensor_scalar(
                  out=dsim3[:, MB - 1:MB, 0:82],
                  in0=dsim3[:, MB - 1:MB, 0:82],
                  scalar1=bk2s_sb[:, 2:3], scalar2=None, op0=OP.mult)

          pend_o82 = []

          def emit_o82(o82ps, qp, pairi, E83):
              for h in range(2):
                  w = WH[(qp, h)]
                  nc.tensor.matmul(
                      o82ps[h][:, 0:w],
                      dsim3[:, 2 * pairi:2 * pairi + 2, 0:82],
                      E83[:, :, h * 512:h * 512 + w],
                      start=(pairi == 0), stop=(pairi == 15), perf_mode=DR)

          unit_ctr = [0]

          def emit_pair(psJ, o82ps, qp, pairi, last=False, pool3=None):
              """Emit psL+exp for pair `pairi`; the o82 accumulation is
              emitted one pair late so PE never stalls waiting on exp."""
              qw = QW[qp]
              E8 = wpool.tile([128, 2048], FP8, tag="E8", bufs=4,
                              name=f"E8_{qp}_{pairi}")
              E83 = E8.rearrange("p (t n) -> p t n", t=2)
              for j in range(2):
                  mb = 2 * pairi + j
                  unit_ctr[0] += 1
                  if pool3 is not None and unit_ctr[0] % 3 == 2:
                      psL = pool3.tile([128, 1024], F32, tag="psL3",
                                       name=f"psL_{qp}_{mb}")
                  else:
                      psL = psJ.tile([128, 1024], F32, tag="psL",
                                     name=f"psL_{qp}_{mb}")
                  for h in range(2):
                      w = WH[(qp, h)]
                      nc.tensor.matmul(
                          psL[:, h * 512:h * 512 + w],
                          qk3[:, :, mb * 128:mb * 128 + 128],
                          qk3[:, :, qp * 1024 + h * 512:
                              qp * 1024 + h * 512 + w],
                          start=True, stop=True, perf_mode=DR)
                  dst = E8[:, j * 1024:j * 1024 + qw]
                  if (qp, mb) in DVE_EXP:
                      nc.vector.tensor_scalar(
                          out=dst.bitcast(U8), in0=psL[:, 0:qw],
                          scalar1=float(A_SCH), scalar2=float(B_SCH),
                          op0=OP.mult, op1=OP.add)
                  else:
                      nc.scalar.activation(dst, psL[:, 0:qw], AF.Exp,
                                           scale=EXP_SCALE)
              pend_o82.append((pairi, E83))
              while len(pend_o82) > (0 if last else 1):
                  pi, e83 = pend_o82.pop(0)
                  emit_o82(o82ps, qp, pi, e83)

          def _cp(engine, out, in_):
              if engine == "act":
                  nc.scalar.copy(out, in_)
              else:
                  nc.vector.tensor_copy(out, in_)

          def tail_steps(psT, o82ps, qp, h, eng):
              """Tail for one 512-query column block as a list of small
              emission steps, so callers can interleave them between pairs
              and the in-order engine queues never stall on a long chain."""
              def _t(name):
                  return psT.tile([128, 512], F32, tag="tail", name=name)
              qc = qp * 2 + h
              st = {}

              def s_rc():
                  st["o82"] = wpool.tile([82, 512], F32R, tag="o82sb",
                                         bufs=2, name=f"o82_{qc}")
                  _cp(eng, st["o82"][:], o82ps[h][:])
                  st["rc"] = wpool.tile([1, 512], F32R, tag="rc", bufs=2,
                                        name=f"rc_{qc}")
                  nc.vector.reciprocal(st["rc"][:], o82ps[h][0:1, :])

              def s_bc():
                  bps = _t(f"bps_{qc}")
                  nc.tensor.matmul(bps[:], ones_sb[0:1, 0:128], st["rc"][:],
                                   start=True, stop=True)
                  st["bc"] = wpool.tile([128, 512], F32, tag="bc", bufs=2,
                                        name=f"bc_{qc}")
                  _cp(eng, st["bc"][:], bps[:])
                  st["ctx"] = wpool.tile([128, 2 * 512], F32R, tag="ctx",
                                         bufs=2, name=f"ctx_{qc}")

              def s_ctx(c2):
                  def f():
                      cps = _t(f"cps_{qc}_{c2}")
                      nc.tensor.matmul(cps[:],
                                       v2t_sb[:, c2 * 128:(c2 + 1) * 128],
                                       st["o82"][0:82, :],
                                       start=True, stop=True)
                      nc.vector.tensor_tensor(
                          st["ctx"][:, c2 * 512:(c2 + 1) * 512],
                          cps[:], st["bc"][:], op=OP.mult)
                  return f

              def s_ob(ob):
                  def f():
                      ops_ = _t(f"ops_{qc}_{ob}")
                      for cc in range(2):
                          nc.tensor.matmul(
                              ops_[:],
                              wws_sb[:, cc * CO + ob * 128:
                                     cc * CO + ob * 128 + 128],
                              st["ctx"][:, cc * 512:(cc + 1) * 512],
                              start=(cc == 0), stop=False)
                      nc.tensor.matmul(
                          ops_[:], wu_sb[:, ob * 128:(ob + 1) * 128],
                          ones_sb[:], start=False, stop=True)
                      outb = wpool.tile([128, 512], F32, tag="outb", bufs=8,
                                        name=f"outb_{qc}_{ob}")
                      _cp(eng if ob % 2 == 0 else
                          ("dve" if eng == "act" else "act"),
                          outb[:], ops_[:])
                      nc.sync.dma_start(
                          out=out_d[ob * 128:(ob + 1) * 128,
                                    qc * 512:(qc + 1) * 512],
                          in_=outb[:])
                  return f

              return [s_rc, s_bc, s_ctx(0), s_ctx(1),
                      s_ob(0), s_ob(1), s_ob(2), s_ob(3)]

          def emit_tail_final(psT, psJ, o82ps, qp):
              """Last tail: both column-blocks interleaved, 4 psum slots
              (psT's 2 plus the now-idle psJ's 2)."""
              slot_i = [0]

              def _slot(name):
                  slot_i[0] += 1
                  if slot_i[0] % 2 == 0:
                      return psT.tile([128, 512], F32, tag="tail", name=name)
                  t = psJ.tile([128, 1024], F32, tag="psL", name=name)
                  return t

              qcs = [qp * 2, qp * 2 + 1]
              ws = [WH[(qp, 0)], WH[(qp, 1)]]
              rcs, o82s, bcs, ctxs = [], [], [], []
              for h in range(2):
                  rc = wpool.tile([1, 512], F32R, tag="rc", bufs=2,
                                  name=f"rc_{qcs[h]}")
                  nc.vector.reciprocal(rc[:, 0:ws[h]],
                                       o82ps[h][0:1, 0:ws[h]])
                  rcs.append(rc)
              for h in range(2):
                  o82 = wpool.tile([82, 512], F32R, tag="o82sb", bufs=2,
                                   name=f"o82_{qcs[h]}")
                  nc.scalar.copy(o82[:, 0:ws[h]], o82ps[h][:, 0:ws[h]])
                  o82s.append(o82)
              bpss = []
              for h in range(2):
                  bps = _slot(f"bps_{qcs[h]}")
                  nc.tensor.matmul(bps[:, 0:ws[h]], ones_sb[0:1, 0:128],
                                   rcs[h][:, 0:ws[h]],
                                   start=True, stop=True)
                  bpss.append(bps)
              for h in range(2):
                  bc = wpool.tile([128, 512], F32, tag="bc", bufs=2,
                                  name=f"bc_{qcs[h]}")
                  _cp("act" if h == 0 else "dve", bc[:, 0:ws[h]],
                      bpss[h][:, 0:ws[h]])
                  bcs.append(bc)
              for h in range(2):
                  ctx = wpool.tile([128, 2 * 512], F32R, tag="ctx", bufs=2,
                                   name=f"ctx_{qcs[h]}")
                  ctxs.append(ctx)
              for c2 in range(2):
                  for h in range(2):
                      cps = _slot(f"cps_{qcs[h]}_{c2}")
                      nc.tensor.matmul(cps[:, 0:ws[h]],
                                       v2t_sb[:, c2 * 128:(c2 + 1) * 128],
                                       o82s[h][0:82, 0:ws[h]],
                                       start=True, stop=True)
                      nc.vector.tensor_tensor(
                          ctxs[h][:, c2 * 512:c2 * 512 + ws[h]],
                          cps[:, 0:ws[h]], bcs[h][:, 0:ws[h]], op=OP.mult)
              for ob in range(4):
                  for h in range(2):
                      qc = qcs[h]
                      w = ws[h]
                      ops_ = _slot(f"ops_{qc}_{ob}")
                      for cc in range(2):
                          nc.tensor.matmul(
                              ops_[:, 0:w],
                              wws_sb[:, cc * CO + ob * 128:
                                     cc * CO + ob * 128 + 128],
                              ctxs[h][:, cc * 512:cc * 512 + w],
                              start=(cc == 0), stop=False)
                      nc.tensor.matmul(
                          ops_[:, 0:w], wu_sb[:, ob * 128:(ob + 1) * 128],
                          ones_sb[0:1, 0:w], start=False, stop=True)
                      outb = wpool.tile([128, 512], F32, tag="outb", bufs=8,
                                        name=f"outb_{qc}_{ob}")
                      _cp("act" if (ob + h) % 2 == 0 else "dve",
                          outb[:, 0:w], ops_[:, 0:w])
                      nc.sync.dma_start(
                          out=out_d[ob * 128:(ob + 1) * 128,
                                    qc * 512:qc * 512 + w],
                          in_=outb[:, 0:w])

          # ---------- emission schedule ----------
          with tc.tile_pool(name="psJ", bufs=2, space="PSUM") as psJ, \
               tc.tile_pool(name="psO", bufs=2, space="PSUM") as psO:
              o82_qp0 = [psO.tile([82, 512], F32, tag="o82",
                                  name=f"o82ps_0_{h}") for h in range(2)]
              with tc.tile_pool(name="psHead", bufs=2, space="PSUM") as psH:
                  emit_loads_early()
                  emit_conv(psH, 0)
                  emit_conv(psH, 1)
                  emit_pooled_path(psH)
                  emit_lg2(psH, 0)
                  emit_lg2(psH, 1)
                  # lag-one interleave: after slab s, pairs 2(s-1), 2(s-1)+1
                  for s in range(2, 8):
                      emit_pair(psJ, o82_qp0, 0, 2 * (s - 2))
                      emit_pair(psJ, o82_qp0, 0, 2 * (s - 2) + 1)
                      emit_conv(psH, s)
                      emit_lg2(psH, s)
                  emit_mask()
                  emit_pair(psJ, o82_qp0, 0, 12)
                  emit_pair(psJ, o82_qp0, 0, 13)
                  emit_tail_consts(psH)
                  emit_pair(psJ, o82_qp0, 0, 14)
                  emit_pair(psJ, o82_qp0, 0, 15, last=True)
              with tc.tile_pool(name="psT", bufs=2, space="PSUM") as psT:
                  o82_qp1 = [psO.tile([82, 512], F32, tag="o82",
                                      name=f"o82ps_1_{h}") for h in range(2)]
                  # qp0's tail rides between qp1 pairs, one small step per
                  # pair, so the in-order ACT/DVE queues never stall on it
                  steps = []
                  q0steps = [tail_steps(psT, o82_qp0, 0, 0, "act"),
                             tail_steps(psT, o82_qp0, 0, 1, "dve")]
                  for i in range(8):
                      steps.append(q0steps[0][i])
                      steps.append(q0steps[1][i])
                  for pairi in range(16):
                      emit_pair(psJ, o82_qp1, 1, pairi, last=(pairi == 15))
                      if pairi >= 1:
                          for _ in range(2):
                              if steps:
                                  steps.pop(0)()
                  while steps:
                      steps.pop(0)()
                  emit_tail_final(psT, psJ, o82_qp1, 1)

    nc.finalize()
    return nc


def _get_program(reps=1):
    if ("nc", reps) not in _CACHE:
        _CACHE[("nc", reps)] = _build_program(reps)
    return _CACHE[("nc", reps)]


def _host_inputs(data_input, Wk, bk, gamma, beta, Wv, bv, Wv2, bv2, Ww, bw):
    f = np.float32
    for name, bias in (("bv", bv), ("bv2", bv2), ("bw", bw)):
        if not np.allclose(np.asarray(bias), 0.0):
            raise NotImplementedError(f"{name} != 0 not supported")
    s = (np.asarray(gamma, f) / np.sqrt(f(1.0) + f(1e-5))).astype(f)
    wk_s = (np.asarray(Wk, f) * s[:, None]) * f(SK)     # [CI, CIN]
    bk2s = ((np.asarray(bk, f) * s + np.asarray(beta, f)) * f(SK)).astype(f)

    # wk8 packed layout: [p, blk*512 + pair*256 + slot*128 + oc]
    # cin = pair*256 + slot*128 + p ; oc_global = blk*128 + oc
    wk8 = np.zeros((128, 1024), NPF8)
    wkT = np.ascontiguousarray(wk_s.T)                  # [CIN, CI]
    for blk in range(2):
        for pr in range(2):
            for sl in range(2):
                cin0 = pr * 256 + sl * 128
                col0 = blk * 512 + pr * 256 + sl * 128
                wk8[:, col0:col0 + 128] = wkT[
                    cin0:cin0 + 128, blk * 128:blk * 128 + 128].astype(NPF8)

    wvT = np.ascontiguousarray(np.asarray(Wv, f).T)
    wv2T = np.ascontiguousarray((np.asarray(Wv2, f) / f(49.0)).T)
    wvO = np.ascontiguousarray(np.asarray(Wv, f))
    wws = np.ascontiguousarray(np.asarray(Ww, f).T * f(SS / SS2))
    xs = np.ascontiguousarray(np.asarray(data_input, f).reshape(B, CIN, N))
    ones1 = np.ones((1, 512), f)
    c8 = np.zeros((128, 256), NPF8)
    c8[:, 0:128] = NPF8(SS)
    fz = np.zeros((128, 8), f)
    fz[0, 0] = 1.0
    x8s = [np.ascontiguousarray(xs[b].astype(NPF8)) for b in range(B)]
    xpools = []
    for b in range(B):
        xp = np.zeros((CIN, 82), f)
        xp[:, 1:] = xs[b].reshape(CIN, 9, 7, 9, 7).sum(axis=(2, 4)).reshape(
            CIN, KK)
        xpools.append(xp)
    bk2p = np.zeros((128, 4), f)
    bk2p[:, 0:2] = bk2s.reshape(2, 128).T
    bk2p[0, 2] = 1.0

    in_maps = []
    for c in range(8):
        b = c % 4
        q0 = (c // 4) * Q0STEP
        xr = np.ascontiguousarray(np.roll(x8s[b], -q0, axis=1))
        in_maps.append({
            "x8": xr, "xpool": xpools[b], "wk8": wk8, "wvT": wvT,
            "wv2T": wv2T, "wvO": wvO, "wws": wws, "bk2s": bk2p,
            "ones1": ones1, "c8ones": c8, "fzero": fz,
        })
    return in_maps


def kernel(data_input, Wk, bk, gamma, beta, Wv, bv, Wv2, bv2, Ww, bw):
    f = np.float32
    in_maps = _host_inputs(data_input, Wk, bk, gamma, beta, Wv, bv, Wv2,
                           bv2, Ww, bw)
    nc = _get_program()
    res = run_bass_kernel_spmd(nc, in_maps, list(range(8)))

    full = np.empty((B, CO, N), f)
    for b in range(B):
        full[b, :, :Q0STEP] = res.results[b]["out"][:, :Q0STEP]
        full[b, :, Q0STEP:] = res.results[4 + b]["out"][:, :QCNT]
    return full.reshape(B, CO, H, W)


# revision 32
# speedup vs baseline: 1.0066x; 1.0066x over previous
"""Trainium2 Bass kernel for EmbededNonLocalLayer (fp8 DoubleRow version).

Distribution: 8 cores = 4 batches x 2 query-halves. Each core holds its
batch's full keys; its query half sits at columns [0:2048) of a rolled x.

Math (per core), with host scales SK=16 (qk path), SW=64 (w2 path),
SS=32 (r1 ones), SS2=1024 (centered simv):
  qk8   = fp8(wk8^T x8 + bk2*SK)              [256, 4096]  (conv, DoubleRow)
  v2    = Wv2 @ (Wv @ xpool)/49               [256, 82]    (fp32, col0=0)
  w28   = fp8(Wv^T v2 * SW)                   [512, 82]    (val^T v2 == x^T w2)
  lgt2  = x8^T w28                            per 128-key block (DoubleRow)
  simv  = softmax_k(lgt2 * S/SW); dsimv8 = fp8((simv - 1/81)*SS2), col0 = SS
  E8    = fp8(exp(qk8^T qk8 * S/SK^2))        ACT exp or DVE Schraudolph bits
  o82   = dsimv8^T E8  (DoubleRow, PSUM accum over 32 key blocks)
          row0 = SS*r1 (softmax denom), rows 1:82 = SS2 * (dsimv^T E)
  ctx   = (v2t^T o82) * (1/row0 bcast);  out = (Ww*SS/SS2)^T ctx + wu^T x 1s
          (wu = Ww @ v2.sum/81 restores the centered-simv mean term)
"""

import sys

sys.path.insert(0, "/opt/trn_rl_repo")

import numpy as np
import ml_dtypes

import concourse.bacc as bacc
import concourse.bass as bass
import concourse.mybir as mybir
from concourse.bass_utils import run_bass_kernel_spmd
from concourse.tile import TileContext

F32 = mybir.dt.float32
F32R = mybir.dt.float32r
FP8 = mybir.dt.float8e4
U8 = mybir.dt.uint8
AF = mybir.ActivationFunctionType
AX = mybir.AxisListType
DR = mybir.MatmulPerfMode.DoubleRow
OP = mybir.AluOpType
NPF8 = ml_dtypes.float8_e4m3

B, CIN, H, W = 4, 512, 63, 63
N = H * W            # 3969
NPAD = 4096
CI, CO = 256, 512
KK = 81
SCALE = 0.0625       # 1/sqrt(CI)
QCNT = 1985
QP = 2048
Q0STEP = 1984
MB = NPAD // 128     # 32 key blocks
SLABS = 8            # 512-column x slabs

SK = 16.0            # qk fp8 scale
SW = 64.0            # w2 fp8 scale
SS = 32.0            # ones column scale (r1 row)
SS2 = 1024.0         # centered-simv scale
EXP_SCALE = SCALE / (SK * SK)
LG2_SCALE = SCALE / SW
SIGMA = 0.35
A_SCH = 8.0 / np.log(2.0) * EXP_SCALE
B_SCH = 8.0 * 7.0 + SIGMA
A16_SCH = 128.0 / np.log(2.0) * LG2_SCALE
B16_SCH = 128.0 * 127.0 + 3.0

# query-column widths per (qp, h): qp1's second half holds only the 449
# real query columns (1985 total); the out tail beyond QCNT is never read.
WH = {(0, 0): 512, (0, 1): 512, (1, 0): 512, (1, 1): 450}
QW = {0: 1024, 1: 962}

# (qp, mb) units whose exp runs on DVE via Schraudolph bits; rest on ACT.
DVE_EXP = {(qp, mb) for qp in range(2) for mb in range(MB)
           if mb % 5 in (1, 3)}

_CACHE = {}


def _build_program(reps=1):
    nc = bacc.Bacc()

    x8_d = nc.dram_tensor("x8", [CIN, N], FP8, kind="ExternalInput")
    xp_d = nc.dram_tensor("xpool", [CIN, 82], F32R, kind="ExternalInput")
    wk8_d = nc.dram_tensor("wk8", [128, 1024], FP8, kind="ExternalInput")
    wv_d = nc.dram_tensor("wvT", [CIN, CI], F32R, kind="ExternalInput")
    wv2_d = nc.dram_tensor("wv2T", [CI, CI], F32R, kind="ExternalInput")
    wvO_d = nc.dram_tensor("wvO", [CI, CIN], F32R, kind="ExternalInput")
    wws_d = nc.dram_tensor("wws", [CI, CO], F32R, kind="ExternalInput")
    bk2s_d = nc.dram_tensor("bk2s", [128, 4], F32, kind="ExternalInput")
    ones_d = nc.dram_tensor("ones1", [1, 512], F32R, kind="ExternalInput")
    c8_d = nc.dram_tensor("c8ones", [128, 256], FP8, kind="ExternalInput")
    fz_d = nc.dram_tensor("fzero", [128, 8], F32R, kind="ExternalInput")
    out_d = nc.dram_tensor("out", [CO, QP], F32, kind="ExternalOutput")

    with TileContext(nc) as tc, \
         nc.allow_low_precision(reason="fp8 attention validated numerically"):
      for _rep in range(reps):
        with tc.tile_pool(name=f"const{_rep}", bufs=1) as cpool, \
             tc.tile_pool(name=f"work{_rep}", bufs=1) as wpool:
          ones_sb = cpool.tile([1, 512], F32R)
          wk8_sb = cpool.tile([128, 1024], FP8)
          wv_sb = cpool.tile([128, 4 * CI], F32R)
          wv2_sb = cpool.tile([128, 2 * CI], F32R)
          wvO_sb = cpool.tile([128, 2 * CIN], F32R)
          wws_sb = cpool.tile([128, 2 * CO], F32R)
          bk2s_sb = cpool.tile([128, 4], F32)
          xp_sb = cpool.tile([128, 4 * 82], F32R)
          x8_sb = cpool.tile([128, 4 * NPAD], FP8)
          qk8_sb = cpool.tile([128, 2 * NPAD], FP8)
          dsimv8_sb = cpool.tile([128, MB * 96], FP8)
          pooled_sb = cpool.tile([128, 2 * 82], F32R)
          v2_sb = cpool.tile([128, 2 * 82], F32R)
          v2t_sb = cpool.tile([82, CI], F32R)
          w28_sb = cpool.tile([128, 4 * 96], FP8)
          r2_sb = cpool.tile([128, MB], F32)
          r2i2_sb = cpool.tile([128, MB], F32)
          v2s_sb = cpool.tile([128, 4], F32R)
          wu_sb = cpool.tile([1, CO], F32R)

          x8_4 = x8_sb.rearrange("p (c n) -> p c n", c=4)
          qk3 = qk8_sb.rearrange("p (t n) -> p t n", t=2)
          w28_4 = w28_sb.rearrange("p (c k) -> p c k", c=4)  # k=96
          dsim3 = dsimv8_sb.rearrange("p (m c) -> p m c", m=MB)  # c=96

          # ---------- emission helpers ----------
          def _slab_dma(s):
              n0 = s * 512
              rl = min(512, N - n0)
              nc.sync.dma_start(
                  out=x8_4[:, :, n0:n0 + rl],
                  in_=x8_d.rearrange("(c p) n -> p c n", c=4)[:, :, n0:n0 + rl])

          def emit_loads_early():
              nc.sync.dma_start(out=wk8_sb[:], in_=wk8_d[:])
              _slab_dma(0)
              _slab_dma(1)
              nc.sync.dma_start(out=bk2s_sb[:], in_=bk2s_d[:])
              _slab_dma(2)
              _slab_dma(3)
              nc.sync.dma_start(
                  out=xp_sb.rearrange("p (c k) -> p c k", c=4),
                  in_=xp_d.rearrange("(c p) k -> p c k", c=4))
              nc.sync.dma_start(
                  out=wv_sb.rearrange("p (c k) -> p c k", c=4),
                  in_=wv_d.rearrange("(c p) k -> p c k", c=4))
              nc.sync.dma_start(
                  out=wv2_sb.rearrange("p (c k) -> p c k", c=2),
                  in_=wv2_d.rearrange("(c p) k -> p c k", c=2))
              nc.sync.dma_start(
                  out=wvO_sb.rearrange("p (c k) -> p c k", c=2),
                  in_=wvO_d.rearrange("(c p) k -> p c k", c=2))
              nc.sync.dma_start(
                  out=wws_sb.rearrange("p (c k) -> p c k", c=2),
                  in_=wws_d.rearrange("(c p) k -> p c k", c=2))
              nc.sync.dma_start(out=ones_sb[:], in_=ones_d[:])
              nc.sync.dma_start(
                  out=dsim3[:, :, 0:1],
                  in_=c8_d[:, 0:MB].rearrange("p (m c) -> p m c", m=MB))
              for s in range(4, 8):
                  _slab_dma(s)
              for cc in range(4):
                  nc.sync.dma_start(
                      out=x8_4[:, cc:cc + 1, N:NPAD],
                      in_=c8_d[:, 128:128 + (NPAD - N)].rearrange(
                          "p (o n) -> p o n", o=1))

          def emit_conv(psH, s):
              n0 = s * 512
              for blk in range(2):
                  ps = psH.tile([128, 512], F32, tag="ps", name=f"cv{s}_{blk}")
                  for pr in range(2):
                      lhsT = wk8_sb[:, blk * 512 + pr * 256:
                                    blk * 512 + pr * 256 + 256].rearrange(
                          "p (s o) -> p s o", s=2)
                      nc.tensor.matmul(
                          ps[:], lhsT,
                          x8_4[:, 2 * pr:2 * pr + 2, n0:n0 + 512],
                          start=(pr == 0), stop=(pr == 1), perf_mode=DR)
                  qslice = qk8_sb[:, blk * NPAD + n0:blk * NPAD + n0 + 512]
                  if blk == 0:
                      nc.scalar.activation(qslice, ps[:], AF.Identity,
                                           bias=bk2s_sb[:, blk:blk + 1])
                  else:
                      nc.vector.tensor_scalar(
                          out=qslice, in0=ps[:],
                          scalar1=bk2s_sb[:, blk:blk + 1], scalar2=None,
                          op0=OP.add)

          def emit_pooled_path(psH):
              for blk in range(2):
                  ps = psH.tile([128, 512], F32, tag="ps", name=f"pooled{blk}")
                  for cc in range(4):
                      nc.tensor.matmul(
                          ps[:, :82],
                          wv_sb[:, cc * CI + blk * 128:
                                cc * CI + blk * 128 + 128],
                          xp_sb[:, cc * 82:(cc + 1) * 82],
                          start=(cc == 0), stop=(cc == 3))
                  nc.vector.tensor_copy(pooled_sb[:, blk * 82:(blk + 1) * 82],
                                        ps[:, :82])
              for blk in range(2):
                  ps = psH.tile([128, 512], F32, tag="ps", name=f"v2_{blk}")
                  for cc in range(2):
                      nc.tensor.matmul(
                          ps[:, :82],
                          wv2_sb[:, cc * CI + blk * 128:
                                 cc * CI + blk * 128 + 128],
                          pooled_sb[:, cc * 82:(cc + 1) * 82],
                          start=(cc == 0), stop=(cc == 1))
                  nc.vector.tensor_copy(v2_sb[:, blk * 82:(blk + 1) * 82],
                                        ps[:, :82])
              # w2 = Wv^T v2 (contract ci), scaled into fp8
              for oc4 in range(4):
                  ps = psH.tile([128, 512], F32, tag="ps", name=f"w2_{oc4}")
                  for cc in range(2):
                      nc.tensor.matmul(
                          ps[:, :82],
                          wvO_sb[:, cc * CIN + oc4 * 128:
                                 cc * CIN + oc4 * 128 + 128],
                          v2_sb[:, cc * 82:(cc + 1) * 82],
                          start=(cc == 0), stop=(cc == 1))
                  nc.vector.tensor_scalar(
                      out=w28_sb[:, oc4 * 96:oc4 * 96 + 82], in0=ps[:, :82],
                      scalar1=SW, scalar2=None, op0=OP.mult)

          def emit_tail_consts(psH):
              ps = psH.tile([128, 512], F32, tag="ps", name="v2t")
              for cc in range(2):
                  nc.tensor.matmul(ps[:82, :CI],
                                   pooled_sb[:, cc * 82:(cc + 1) * 82],
                                   wv2_sb[:, cc * CI:(cc + 1) * CI],
                                   start=(cc == 0), stop=(cc == 1))
              nc.vector.tensor_copy(v2t_sb[:], ps[:82, :CI])
              # v2s = rowsum(v2) * SS2/(SS*81); wu = v2s^T wws  -> [1, CO]
              # v2s is stored interleaved [c0, 0, c1, 0] so the wu matmul's
              # stationary free extent is 2 (fp32r needs an even count)
              nc.sync.dma_start(out=v2s_sb[:], in_=fz_d[:, 4:8])
              nc.vector.reduce_sum(
                  v2s_sb.rearrange("p (c z) -> p c z", c=2)[:, :, 0:1],
                  v2_sb.rearrange("p (c k) -> p c k", c=2),
                  axis=AX.X)
              nc.vector.tensor_scalar(
                  out=v2s_sb[:], in0=v2s_sb[:],
                  scalar1=float(SS2 / (SS * 81.0)), scalar2=None, op0=OP.mult)
              ps = psH.tile([128, 512], F32, tag="ps", name="wu")
              for cc in range(2):
                  nc.tensor.matmul(ps[0:2, :CO],
                                   v2s_sb[:, 2 * cc:2 * cc + 2],
                                   wws_sb[:, cc * CO:(cc + 1) * CO],
                                   start=(cc == 0), stop=(cc == 1))
              nc.vector.tensor_copy(wu_sb[:], ps[0:1, :CO])

          def emit_lg2(psH, s):
              ps2 = psH.tile([128, 512], F32, tag="ps", name=f"lg{s}")
              for j in range(4):
                  mb = 4 * s + j
                  for pr in range(2):
                      nc.tensor.matmul(
                          ps2[:, j * 82:j * 82 + 82],
                          x8_4[:, 2 * pr:2 * pr + 2, mb * 128:mb * 128 + 128],
                          w28_4[:, 2 * pr:2 * pr + 2, 0:82],
                          start=(pr == 0), stop=(pr == 1), perf_mode=DR)
              ex2 = wpool.tile([128, 4 * 82], F32, tag="ex2", bufs=2,
                               name=f"ex2_{s}")
              ps2v = ps2[:, 0:328].rearrange("p (g c) -> p g c", g=4)
              ex2v = ex2.rearrange("p (g c) -> p g c", g=4)
              nc.scalar.activation(ex2v[:, :, 1:82], ps2v[:, :, 1:82],
                                   AF.Exp, scale=LG2_SCALE)
              nc.vector.reduce_sum(r2_sb[:, 4 * s:4 * s + 4],
                                   ex2v[:, :, 1:82], axis=AX.X)
              nc.vector.reciprocal(r2i2_sb[:, 4 * s:4 * s + 4],
                                   r2_sb[:, 4 * s:4 * s + 4])
              nc.vector.tensor_scalar(
                  out=r2i2_sb[:, 4 * s:4 * s + 4],
                  in0=r2i2_sb[:, 4 * s:4 * s + 4],
                  scalar1=SS2, scalar2=None, op0=OP.mult)
              for j in range(4):
                  mb = 4 * s + j
                  nc.gpsimd.tensor_scalar(
                      out=dsim3[:, mb:mb + 1, 1:82],
                      in0=ex2v[:, j:j + 1, 1:82],
                      scalar1=r2i2_sb[:, mb:mb + 1],
                      scalar2=float(SS2 / 81.0),
                      op0=OP.mult, op1=OP.subtract)

          def emit_mask():
              nc.gpsimd.tensor_scalar(
                  out=dsim3[:, MB - 1:MB, 0:82],
                  in0=dsim3[:, MB - 1:MB, 0:82],
                  scalar1=bk2s_sb[:, 2:3], scalar2=None, op0=OP.mult)

          pend_o82 = []

          def emit_o82(o82ps, qp, pairi, E83):
              for h in range(2):
                  w = WH[(qp, h)]
                  nc.tensor.matmul(
                      o82ps[h][:, 0:w],
                      dsim3[:, 2 * pairi:2 * pairi + 2, 0:82],
                      E83[:, :, h * 512:h * 512 + w],
                      start=(pairi == 0), stop=(pairi == 15), perf_mode=DR)

          unit_ctr = [0]

          def emit_pair(psJ, o82ps, qp, pairi, last=False, pool3=None):
              """Emit psL+exp for pair `pairi`; the o82 accumulation is
              emitted one pair late so PE never stalls waiting on exp."""
              qw = QW[qp]
              E8 = wpool.tile([128, 2048], FP8, tag="E8", bufs=4,
                              name=f"E8_{qp}_{pairi}")
              E83 = E8.rearrange("p (t n) -> p t n", t=2)
              for j in range(2):
                  mb = 2 * pairi + j
                  unit_ctr[0] += 1
                  if pool3 is not None and unit_ctr[0] % 3 == 2:
                      psL = pool3.tile([128, 1024], F32, tag="psL3",
                                       name=f"psL_{qp}_{mb}")
                  else:
                      psL = psJ.tile([128, 1024], F32, tag="psL",
                                     name=f"psL_{qp}_{mb}")
                  for h in range(2):
                      w = WH[(qp, h)]
                      nc.tensor.matmul(
                          psL[:, h * 512:h * 512 + w],
                          qk3[:, :, mb * 128:mb * 128 + 128],
                          qk3[:, :, qp * 1024 + h * 512:
                              qp * 1024 + h * 512 + w],
                          start=True, stop=True, perf_mode=DR)
                  dst = E8[:, j * 1024:j * 1024 + qw]
                  if (qp, mb) in DVE_EXP:
                      nc.vector.tensor_scalar(
                          out=dst.bitcast(U8), in0=psL[:, 0:qw],
                          scalar1=float(A_SCH), scalar2=float(B_SCH),
                          op0=OP.mult, op1=OP.add)
                  else:
                      nc.scalar.activation(dst, psL[:, 0:qw], AF.Exp,
                                           scale=EXP_SCALE)
              pend_o82.append((pairi, E83))
              while len(pend_o82) > (0 if last else 1):
                  pi, e83 = pend_o82.pop(0)
                  emit_o82(o82ps, qp, pi, e83)

          def _cp(engine, out, in_):
              if engine == "act":
                  nc.scalar.copy(out, in_)
              else:
                  nc.vector.tensor_copy(out, in_)

          def emit_tail_h(psT, o82ps, qp, h, eng):
              """Tail for one 512-query column block; copies on `eng`."""
              def _t(name):
                  return psT.tile([128, 512], F32, tag="tail", name=name)
              qc = qp * 2 + h
              o82 = wpool.tile([82, 512], F32R, tag="o82sb", bufs=2,
                               name=f"o82_{qc}")
              _cp(eng, o82[:], o82ps[h][:])
              rc = wpool.tile([1, 512], F32R, tag="rc", bufs=2,
                              name=f"rc_{qc}")
              nc.vector.reciprocal(rc[:], o82ps[h][0:1, :])
              bps = _t(f"bps_{qc}")
              nc.tensor.matmul(bps[:], ones_sb[0:1, 0:128], rc[:],
                               start=True, stop=True)
              bc = wpool.tile([128, 512], F32, tag="bc", bufs=2,
                              name=f"bc_{qc}")
              _cp(eng, bc[:], bps[:])
              ctx = wpool.tile([128, 2 * 512], F32R, tag="ctx", bufs=2,
                               name=f"ctx_{qc}")
              for c2 in range(2):
                  cps = _t(f"cps_{qc}_{c2}")
                  nc.tensor.matmul(cps[:],
                                   v2t_sb[:, c2 * 128:(c2 + 1) * 128],
                                   o82[0:82, :], start=True, stop=True)
                  # fold the softmax normalization into ctx
                  nc.vector.tensor_tensor(ctx[:, c2 * 512:(c2 + 1) * 512],
                                          cps[:], bc[:], op=OP.mult)
              for ob in range(4):
                  ops_ = _t(f"ops_{qc}_{ob}")
                  for cc in range(2):
                      nc.tensor.matmul(
                          ops_[:],
                          wws_sb[:, cc * CO + ob * 128:
                                 cc * CO + ob * 128 + 128],
                          ctx[:, cc * 512:(cc + 1) * 512],
                          start=(cc == 0), stop=False)
                  # rank-1 mean restore: + wu^T x ones
                  nc.tensor.matmul(
                      ops_[:], wu_sb[:, ob * 128:(ob + 1) * 128],
                      ones_sb[:], start=False, stop=True)
                  outb = wpool.tile([128, 512], F32, tag="outb", bufs=8,
                                    name=f"outb_{qc}_{ob}")
                  _cp(eng if ob % 2 == 0 else
                      ("dve" if eng == "act" else "act"), outb[:], ops_[:])
                  nc.sync.dma_start(
                      out=out_d[ob * 128:(ob + 1) * 128,
                                qc * 512:(qc + 1) * 512],
                      in_=outb[:])

          def emit_tail_final(psT, psJ, o82ps, qp):
              """Last tail: both column-blocks interleaved, 4 psum slots
              (psT's 2 plus the now-idle psJ's 2)."""
              slot_i = [0]

              def _slot(name):
                  slot_i[0] += 1
                  if slot_i[0] % 2 == 0:
                      return psT.tile([128, 512], F32, tag="tail", name=name)
                  t = psJ.tile([128, 1024], F32, tag="psL", name=name)
                  return t

              qcs = [qp * 2, qp * 2 + 1]
              ws = [WH[(qp, 0)], WH[(qp, 1)]]
              rcs, o82s, bcs, ctxs = [], [], [], []
              for h in range(2):
                  rc = wpool.tile([1, 512], F32R, tag="rc", bufs=2,
                                  name=f"rc_{qcs[h]}")
                  nc.vector.reciprocal(rc[:, 0:ws[h]],
                                       o82ps[h][0:1, 0:ws[h]])
                  rcs.append(rc)
              for h in range(2):
                  o82 = wpool.tile([82, 512], F32R, tag="o82sb", bufs=2,
                                   name=f"o82_{qcs[h]}")
                  nc.scalar.copy(o82[:, 0:ws[h]], o82ps[h][:, 0:ws[h]])
                  o82s.append(o82)
              bpss = []
              for h in range(2):
                  bps = _slot(f"bps_{qcs[h]}")
                  nc.tensor.matmul(bps[:, 0:ws[h]], ones_sb[0:1, 0:128],
                                   rcs[h][:, 0:ws[h]],
                                   start=True, stop=True)
                  bpss.append(bps)
              for h in range(2):
                  bc = wpool.tile([128, 512], F32, tag="bc", bufs=2,
                                  name=f"bc_{qcs[h]}")
                  _cp("act" if h == 0 else "dve", bc[:, 0:ws[h]],
                      bpss[h][:, 0:ws[h]])
                  bcs.append(bc)
              for h in range(2):
                  ctx = wpool.tile([128, 2 * 512], F32R, tag="ctx", bufs=2,
                                   name=f"ctx_{qcs[h]}")
                  ctxs.append(ctx)
              for c2 in range(2):
                  for h in range(2):
                      cps = _slot(f"cps_{qcs[h]}_{c2}")
                      nc.tensor.matmul(cps[:, 0:ws[h]],
                                       v2t_sb[:, c2 * 128:(c2 + 1) * 128],
                                       o82s[h][0:82, 0:ws[h]],
                                       start=True, stop=True)
                      nc.vector.tensor_tensor(
                          ctxs[h][:, c2 * 512:c2 * 512 + ws[h]],
                          cps[:, 0:ws[h]], bcs[h][:, 0:ws[h]], op=OP.mult)
              for ob in range(4):
                  for h in range(2):
                      qc = qcs[h]
                      w = ws[h]
                      ops_ = _slot(f"ops_{qc}_{ob}")
                      for cc in range(2):
                          nc.tensor.matmul(
                              ops_[:, 0:w],
                              wws_sb[:, cc * CO + ob * 128:
                                     cc * CO + ob * 128 + 128],
                              ctxs[h][:, cc * 512:cc * 512 + w],
                              start=(cc == 0), stop=False)
                      nc.tensor.matmul(
                          ops_[:, 0:w], wu_sb[:, ob * 128:(ob + 1) * 128],
                          ones_sb[0:1, 0:w], start=False, stop=True)
                      outb = wpool.tile([128, 512], F32, tag="outb", bufs=8,
                                        name=f"outb_{qc}_{ob}")
                      _cp("act" if (ob + h) % 2 == 0 else "dve",
                          outb[:, 0:w], ops_[:, 0:w])
                      nc.sync.dma_start(
                          out=out_d[ob * 128:(ob + 1) * 128,
                                    qc * 512:qc * 512 + w],
                          in_=outb[:, 0:w])

          # ---------- emission schedule ----------
          with tc.tile_pool(name="psJ", bufs=2, space="PSUM") as psJ, \
               tc.tile_pool(name="psO", bufs=2, space="PSUM") as psO:
              o82_qp0 = [psO.tile([82, 512], F32, tag="o82",
                                  name=f"o82ps_0_{h}") for h in range(2)]
              with tc.tile_pool(name="psHead", bufs=2, space="PSUM") as psH:
                  emit_loads_early()
                  emit_conv(psH, 0)
                  emit_conv(psH, 1)
                  emit_pooled_path(psH)
                  emit_lg2(psH, 0)
                  emit_lg2(psH, 1)
                  # lag-one interleave: after slab s, pairs 2(s-1), 2(s-1)+1
                  for s in range(2, 8):
                      emit_pair(psJ, o82_qp0, 0, 2 * (s - 2))
                      emit_pair(psJ, o82_qp0, 0, 2 * (s - 2) + 1)
                      emit_conv(psH, s)
                      emit_lg2(psH, s)
                  emit_mask()
                  emit_pair(psJ, o82_qp0, 0, 12)
                  emit_pair(psJ, o82_qp0, 0, 13)
                  emit_tail_consts(psH)
                  emit_pair(psJ, o82_qp0, 0, 14)
                  emit_pair(psJ, o82_qp0, 0, 15, last=True)
              with tc.tile_pool(name="psT", bufs=2, space="PSUM") as psT:
                  o82_qp1 = [psO.tile([82, 512], F32, tag="o82",
                                      name=f"o82ps_1_{h}") for h in range(2)]
                  # qp0's tail rides between early qp1 pairs so the in-order
                  # ACT/DVE queues don't stall qp1's exps behind it
                  for pairi in range(3):
                      emit_pair(psJ, o82_qp1, 1, pairi)
                  emit_tail_h(psT, o82_qp0, 0, 0, "act")
                  for pairi in range(3, 6):
                      emit_pair(psJ, o82_qp1, 1, pairi)
                  emit_tail_h(psT, o82_qp0, 0, 1, "dve")
                  for pairi in range(6, 16):
                      emit_pair(psJ, o82_qp1, 1, pairi, last=(pairi == 15))
                  emit_tail_final(psT, psJ, o82_qp1, 1)

    nc.finalize()
    return nc


def _get_program(reps=1):
    if ("nc", reps) not in _CACHE:
        _CACHE[("nc", reps)] = _build_program(reps)
    return _CACHE[("nc", reps)]


def _host_inputs(data_input, Wk, bk, gamma, beta, Wv, bv, Wv2, bv2, Ww, bw):
    f = np.float32
    for name, bias in (("bv", bv), ("bv2", bv2), ("bw", bw)):
        if not np.allclose(np.asarray(bias), 0.0):
            raise NotImplementedError(f"{name} != 0 not supported")
    s = (np.asarray(gamma, f) / np.sqrt(f(1.0) + f(1e-5))).astype(f)
    wk_s = (np.asarray(Wk, f) * s[:, None]) * f(SK)     # [CI, CIN]
    bk2s = ((np.asarray(bk, f) * s + np.asarray(beta, f)) * f(SK)).astype(f)

    # wk8 packed layout: [p, blk*512 + pair*256 + slot*128 + oc]
    # cin = pair*256 + slot*128 + p ; oc_global = blk*128 + oc
    wk8 = np.zeros((128, 1024), NPF8)
    wkT = np.ascontiguousarray(wk_s.T)                  # [CIN, CI]
    for blk in range(2):
        for pr in range(2):
            for sl in range(2):
                cin0 = pr * 256 + sl * 128
                col0 = blk * 512 + pr * 256 + sl * 128
                wk8[:, col0:col0 + 128] = wkT[
                    cin0:cin0 + 128, blk * 128:blk * 128 + 128].astype(NPF8)

    wvT = np.ascontiguousarray(np.asarray(Wv, f).T)
    wv2T = np.ascontiguousarray((np.asarray(Wv2, f) / f(49.0)).T)
    wvO = np.ascontiguousarray(np.asarray(Wv, f))
    wws = np.ascontiguousarray(np.asarray(Ww, f).T * f(SS / SS2))
    xs = np.ascontiguousarray(np.asarray(data_input, f).reshape(B, CIN, N))
    ones1 = np.ones((1, 512), f)
    c8 = np.zeros((128, 256), NPF8)
    c8[:, 0:128] = NPF8(SS)
    fz = np.zeros((128, 8), f)
    fz[0, 0] = 1.0
    x8s = [np.ascontiguousarray(xs[b].astype(NPF8)) for b in range(B)]
    xpools = []
    for b in range(B):
        xp = np.zeros((CIN, 82), f)
        xp[:, 1:] = xs[b].reshape(CIN, 9, 7, 9, 7).sum(axis=(2, 4)).reshape(
            CIN, KK)
        xpools.append(xp)
    bk2p = np.zeros((128, 4), f)
    bk2p[:, 0:2] = bk2s.reshape(2, 128).T
    bk2p[0, 2] = 1.0

    in_maps = []
    for c in range(8):
        b = c % 4
        q0 = (c // 4) * Q0STEP
        xr = np.ascontiguousarray(np.roll(x8s[b], -q0, axis=1))
        in_maps.append({
            "x8": xr, "xpool": xpools[b], "wk8": wk8, "wvT": wvT,
            "wv2T": wv2T, "wvO": wvO, "wws": wws, "bk2s": bk2p,
            "ones1": ones1, "c8ones": c8, "fzero": fz,
        })
    return in_maps


def kernel(data_input, Wk, bk, gamma, beta, Wv, bv, Wv2, bv2, Ww, bw):
    f = np.float32
    in_maps = _host_inputs(data_input, Wk, bk, gamma, beta, Wv, bv, Wv2,
                           bv2, Ww, bw)
    nc = _get_program()
    res = run_bass_kernel_spmd(nc, in_maps, list(range(8)))

    full = np.empty((B, CO, N), f)
    for b in range(B):
        full[b, :, :Q0STEP] = res.results[b]["out"][:, :Q0STEP]
        full[b, :, Q0STEP:] = res.results[4 + b]["out"][:, :QCNT]
    return full.reshape(B, CO, H, W)
